# revision 2
# baseline (speedup 1.0000x reference)
"""Trainium2 Bass kernel for an 8-head self-attention block (MHA), v4.

Problem: x[2, 4096, 512], 8 heads x 64 dims, torch-Linear q/k/v/o projections,
softmax attention, residual:  out = softmax(q k^T / 8) v @ Wo^T + bo + x.

Sharding (8 NeuronCores, no collectives): core c handles batch b = c // 4 and
query rows (c % 4) * 1024 ... + 1024, for ALL heads.  K/V for the full
sequence are computed on every core of a batch group.

Measured HW behavior driving this design (all microbenched on the device):
  - Every matmul instruction costs N_out_cols cycles; fp8 DoubleRow's win
    is K=256 per instruction (half the instructions), not faster columns.
  - K=64 matmuls under sustained multi-engine load drag the PE clock from
    2.4 GHz down to 1.2 GHz (half-idle array -> power management), and the
    whole attention phase runs 2x slow.  K=128 matmuls hold 2.4 GHz under
    identical load.  => scores run K=128 by DUPLICATING each head's 64
    kT/qT dims across all 128 partitions (computes 2x scores; the extra
    2x is folded into the q projection scale).  The duplicates are built
    by partition-shifted SBUF->SBUF DMA copies - no engine time.
  - exp is one instruction per [128, 1024] chunk, alternating ACT (true
    exp) and DVE (Schraudolph bit-trick to fp8e4 bits, ~4% rel err, same
    order as e4m3 quantization of P~), per a tunable pattern.
  - PV flipped + fp8 DoubleRow: stationary [V|1] fp8, moving P~ pair
    [128, 2, 512] => psum o^T[65, q], denominator in row 64, 2 matmuls
    per chunk-pair, running two pairs behind scores.
  - Normalize: DVE copies psum->SBUF (denom row shifted to partition 0 -
    the custom fast-reciprocal needs base-0 operands), reciprocal_approx
    _fast, GpSimd partition_broadcast + multiply -> oT bf16.  oT must be
    bf16 (fp8 oT alone costs ~1e-2 end-to-end error).
  - Output projection bf16 K=64 runs in the tail where ACT/DVE are
    drained and the clock recovers.
"""

import numpy as np

B = 2
S = 4096
E = 512
H = 8
D = 64
P = 128
NJ = S // P          # 32 s-chunks of 128
NJP = NJ // 2        # 16 s-chunk pairs
QR = S // 4          # 1024 query rows per core
NQC = QR // P        # 8 query chunks of 128
NQS = QR // 512      # 2 query strips of 512
NKS = S // 512       # 8 s-strips of 512
FC = E // P          # 4 f-chunks

# Schraudolph fast-exp constants (DVE rounds f32->int to nearest).
A_SCH8 = 8.0 / np.log(2.0)
B_SCH8 = 56.0 - 0.35

# exp engine per chunk ('A' = ACT true exp, 'V' = DVE Schraudolph),
# indexed by j % len.
EXP_PATTERN = "AVAVAVAAAVAVAVAA"

_CACHE = {}


def _build_nc():
    import concourse.bass as bass
    import concourse.tile as tile
    from concourse import bacc, mybir

    f32 = mybir.dt.float32
    bf16 = mybir.dt.bfloat16
    fp8 = mybir.dt.float8e4
    i8 = mybir.dt.int8
    AFT = mybir.ActivationFunctionType
    Alu = mybir.AluOpType
    DR = mybir.MatmulPerfMode.DoubleRow

    nc = bacc.Bacc("TRN2", target_bir_lowering=False, debug=False, num_devices=8)

    xT_d = nc.declare_dram_parameter("xT", [4, P, 2, 2, 1024], fp8, isOutput=False)
    xqT_d = nc.declare_dram_parameter("xqT", [P, 2, 2, QR], fp8, isOutput=False)
    xres_d = nc.declare_dram_parameter("xres", [P, NQC, E], f32, isOutput=False)
    wq_d = nc.declare_dram_parameter("wq", [P, 2, 2, E], fp8, isOutput=False)
    wk_d = nc.declare_dram_parameter("wk", [P, 2, 2, E], fp8, isOutput=False)
    wv_d = nc.declare_dram_parameter("wv", [P, 2, 2, E], fp8, isOutput=False)
    wo_d = nc.declare_dram_parameter("wo", [P, H, E], bf16, isOutput=False)
    bq_d = nc.declare_dram_parameter("bq", [P, FC], f32, isOutput=False)
    bk_d = nc.declare_dram_parameter("bk", [P, FC], f32, isOutput=False)
    bv_d = nc.declare_dram_parameter("bv", [E], f32, isOutput=False)
    out_d = nc.declare_dram_parameter("out", [QR, E], f32, isOutput=True)

    with tile.TileContext(nc) as tc:
        with tc.tile_pool(name="const", bufs=1) as const, \
             tc.tile_pool(name="persist", bufs=1) as persist:

            # ---- constants ----
            wq_sb = const.tile([P, 2, 2, E], fp8)
            wk_sb = const.tile([P, 2, 2, E], fp8)
            wv_sb = const.tile([P, 2, 2, E], fp8)
            wo_sb = const.tile([P, H, E], bf16)
            bq_sb = const.tile([P, FC], f32)
            bk_sb = const.tile([P, FC], f32)
            bv_sb = const.tile([P, E], f32)
            xres_sb = const.tile([P, NQC, E], f32)

            # ---- persistent activations ----
            # duplicated layouts: head h's 64 dims on partitions 0-63 AND
            # 64-127 (K=128 scores hold full PE clock; 2x scores folded
            # into the q scale)
            kT2_sb = [[persist.tile([P, 1024], fp8, name=f"kT2_{h}_{s}")
                       for s in range(4)] for h in range(H)]  # 32 KB/p
            qT2_sb = [persist.tile([P, QR], fp8, name=f"qT2_{h}")
                      for h in range(H)]                      # 8 KB/p
            # [V | 1] fp8: (s-chunk pair, parity, head, 66) - 66 pads the
            # pair stride to 16B alignment for the DR weight load.
            v_sb = persist.tile([P, NJP, 2, H, 66], fp8)      # ~16.5 KB/p
            # o^T duplicated like kT2 so the output projection is K=128
            # (K=64 in the tail ran at the post-load 1.2 GHz clock with
            # serialized weight loads); Wo/2 host-side
            oT_sb = persist.tile([P, H, QR], bf16)            # 32 KB/p

            nc.vector.memset(v_sb[:, :, :, :, 64:65], 1.0)

            for t, d in ((wq_sb, wq_d), (wk_sb, wk_d), (wv_sb, wv_d),
                         (bq_sb, bq_d), (bk_sb, bk_d)):
                nc.sync.dma_start(out=t[:], in_=d[:])
            nc.sync.dma_start(
                out=bv_sb[:],
                in_=bass.AP(tensor=bv_d, offset=0, ap=[[0, P], [1, E]]))

            with tc.tile_pool(name="xtp", bufs=3) as xtp, \
                 tc.tile_pool(name="work", bufs=3) as work, \
                 tc.tile_pool(name="opool", bufs=2) as opool, \
                 tc.tile_pool(name="ps_sc", bufs=3, space="PSUM") as ps_sc, \
                 tc.tile_pool(name="ps_pv", bufs=1, space="PSUM") as ps_pv:

                def dup_fanout(strip_tile, kind, fc, sl, n):
                    """DMA head-halves of a [128, n] projection strip into
                    the duplicated [p, head, seq] layout (partition-shifted
                    SBUF->SBUF copies, no engine time)."""
                    for t, h in ((slice(0, 64), 2 * fc),
                                 (slice(64, 128), 2 * fc + 1)):
                        if kind == "k":
                            d0, d1 = kT2_sb[h][sl.start // 1024], None
                            nc.sync.dma_start(out=d0[0:64, :],
                                              in_=strip_tile[t, 0:n])
                            nc.sync.dma_start(out=d0[64:128, :],
                                              in_=strip_tile[t, 0:n])
                        else:
                            nc.sync.dma_start(out=qT2_sb[h][0:64, sl],
                                              in_=strip_tile[t, 0:n])
                            nc.sync.dma_start(out=qT2_sb[h][64:128, sl],
                                              in_=strip_tile[t, 0:n])

                # ---- phase B: projections (fp8 DoubleRow, K=256/instr) ----
                # 1024-wide strips: bigger ACT evacuations and 2KB-run DMA
                # fanout copies (dup bandwidth paced the old 512 strips)
                xq = xtp.tile([P, 2, 2, QR], fp8, tag="xq", bufs=1)
                nc.sync.dma_start(out=xq[:], in_=xqT_d[:])
                for f in range(FC):
                    pq = ps_sc.tile([P, QR], f32, tag="sc", name="pq")
                    for g in range(2):
                        for hf in range(2):
                            hsl = slice(hf * 512, (hf + 1) * 512)
                            nc.tensor.matmul(
                                pq[:, hsl], wq_sb[:, g, :, f * P:(f + 1) * P],
                                xq[:, g, :, hsl], start=(g == 0),
                                stop=(g == 1), perf_mode=DR,
                                skip_group_check=True)
                    qs_t = work.tile([P, QR], fp8, tag="qs", name="qs", bufs=2)
                    nc.scalar.activation(
                        qs_t[:], pq[:], AFT.Identity,
                        bias=bq_sb[:, f:f + 1], scale=0.0625)
                    dup_fanout(qs_t, "q", f, slice(0, QR), QR)

                for strip in range(4):
                    ssl = slice(strip * 1024, (strip + 1) * 1024)
                    xt = xtp.tile([P, 2, 2, 1024], fp8, tag="xt", bufs=2)
                    nc.sync.dma_start(out=xt[:], in_=xT_d[strip])
                    for f in range(FC):
                        pk = ps_sc.tile([P, 1024], f32, tag="sc", name="pk")
                        for g in range(2):
                            for hf in range(2):
                                hsl = slice(hf * 512, (hf + 1) * 512)
                                nc.tensor.matmul(
                                    pk[:, hsl],
                                    wk_sb[:, g, :, f * P:(f + 1) * P],
                                    xt[:, g, :, hsl], start=(g == 0),
                                    stop=(g == 1), perf_mode=DR,
                                    skip_group_check=True)
                        ks_t = work.tile([P, 1024], fp8, tag="ks", name="ks", bufs=2)
                        nc.scalar.activation(
                            ks_t[:], pk[:], AFT.Identity,
                            bias=bk_sb[:, f:f + 1])
                        dup_fanout(ks_t, "k", f, ssl, 1024)
                    for k in range(8):
                        j = strip * 8 + k
                        pv = ps_sc.tile([P, E], f32, tag="sc", name="pvx")
                        for g in range(2):
                            nc.tensor.matmul(
                                pv[:], xt[:, g, :, k * P:(k + 1) * P],
                                wv_sb[:, g, :, :], start=(g == 0),
                                stop=(g == 1), perf_mode=DR,
                                skip_group_check=True)
                        pv_v = pv[:].rearrange("p (h d) -> p h d", h=H)
                        bv_v = bv_sb[:].rearrange("p (h d) -> p h d", h=H)
                        nc.vector.tensor_add(
                            v_sb[:, j // 2, j % 2, :, 0:64], pv_v[:], bv_v[:])

                # tail-only constants, off the startup DMA critical path
                nc.sync.dma_start(out=wo_sb[:], in_=wo_d[:])
                nc.sync.dma_start(out=xres_sb[:], in_=xres_d[:])

                # ---- phase C: attention ----
                def emit_scores_exp(h, j, pt8):
                    sc = ps_sc.tile([P, QR], f32, tag="sc", name="sc")
                    for hf in range(2):
                        hsl = slice(hf * 512, (hf + 1) * 512)
                        nc.tensor.matmul(
                            sc[:, hsl],
                            kT2_sb[h][j // 8][:, (j % 8) * P:(j % 8 + 1) * P],
                            qT2_sb[h][:, hsl],
                            start=True, stop=True, skip_group_check=True)
                    dst = pt8[:, j % 2, :]
                    if EXP_PATTERN[j % len(EXP_PATTERN)] == "A":
                        nc.scalar.activation(dst, sc[:], AFT.Exp)
                    else:
                        nc.vector.tensor_scalar(
                            dst.bitcast(i8), sc[:], float(A_SCH8),
                            float(B_SCH8), Alu.mult, Alu.add)

                def emit_pv(h, jp, po, pt8):
                    for hf in range(2):
                        hsl = slice(hf * 512, (hf + 1) * 512)
                        nc.tensor.matmul(
                            po[:, hf, :], v_sb[:, jp, :, h, 0:65],
                            pt8[:, :, hsl], start=(jp == 0),
                            stop=(jp == NJP - 1), perf_mode=DR,
                            skip_group_check=True)

                def emit_stage(po, h):
                    stg = opool.tile([D, 2, 512], f32, tag="stg", name="stg", bufs=1)
                    drow = opool.tile([1, 2, 512], f32, tag="drow",
                                      name="drow")
                    nc.vector.tensor_copy(stg[:], po[0:64, :, :])
                    nc.vector.tensor_copy(drow[:], po[64:65, :, :])
                    return (stg, drow)

                def emit_normalize(staged, h, on_dve=False):
                    stg, drow = staged
                    rcp = opool.tile([1, 2, 512], f32, tag="rcp", name="rcp")
                    nc.vector.reciprocal_approx_fast(rcp[:], drow[:])
                    rcpb = opool.tile([D, QR], f32, tag="rcpb", name="rcpb", bufs=1)
                    eng = nc.vector if on_dve else nc.gpsimd
                    for hf in range(2):
                        hsl = slice(hf * 512, (hf + 1) * 512)
                        nc.gpsimd.partition_broadcast(
                            rcpb[:, hsl], rcp[:, hf, :])
                        eng.tensor_mul(
                            oT_sb[0:64, h, hsl], stg[:, hf, :], rcpb[:, hsl])
                    nc.sync.dma_start(out=oT_sb[64:128, h, :],
                                      in_=oT_sb[0:64, h, :])

                pending = None
                for h in range(H):
                    po = ps_pv.tile([65, 2, 512], f32, tag="pv", name="po")
                    fifo = []
                    for jp in range(NJP):
                        pt8 = work.tile([P, 2, QR], fp8, tag="pt8",
                                        name="pt8")
                        emit_scores_exp(h, 2 * jp, pt8)
                        emit_scores_exp(h, 2 * jp + 1, pt8)
                        fifo.append((jp, pt8))
                        if len(fifo) > 2:
                            pj, p8 = fifo.pop(0)
                            emit_pv(h, pj, po, p8)
                        if pending is not None and jp == 4:
                            emit_normalize(*pending)
                            pending = None
                    for pj, p8 in fifo:
                        emit_pv(h, pj, po, p8)
                    pending = (emit_stage(po, h), h)
                emit_normalize(*pending, on_dve=True)

                # ---- phase D: output projection (bf16, K=64/head) ----
                for qc in range(NQC):
                    poq = ps_sc.tile([P, E], f32, tag="sc", name="poq")
                    for h in range(H):
                        nc.tensor.matmul(
                            poq[:], oT_sb[:, h, qc * P:(qc + 1) * P],
                            wo_sb[:, h, :], start=(h == 0),
                            stop=(h == H - 1), skip_group_check=True)
                    ot = opool.tile([P, E], f32, tag="ot", name="ot")
                    nc.vector.tensor_add(ot[:], poq[:], xres_sb[:, qc, :])
                    nc.sync.dma_start(
                        out=out_d[qc * P:(qc + 1) * P, :], in_=ot[:])

    nc.compile()
    return nc


def _get_nc():
    if "nc" not in _CACHE:
        _CACHE["nc"] = _build_nc()
    return _CACHE["nc"]


def run_spmd(in_maps, **kw):
    from concourse.bass_utils import run_bass_kernel_spmd
    nc = _get_nc()
    return run_bass_kernel_spmd(nc, in_maps, list(range(8)), **kw)


def make_in_maps(x, Wq, bq, Wk, bk, Wv, bv, Wo, bo):
    import ml_dtypes
    f8 = ml_dtypes.float8_e4m3
    bf = ml_dtypes.bfloat16
    x = np.asarray(x, dtype=np.float32)
    f32c = lambda a: np.ascontiguousarray(np.asarray(a, dtype=np.float32))
    f8c = lambda a: np.ascontiguousarray(
        np.asarray(a, dtype=np.float32).astype(f8))

    # x^T in fp8 DoubleRow pair layout [p, g, t, s]: e = (2g+t)*128 + p
    def dr_x(xb):  # xb [rows, E] -> [128, 2, 2, rows]
        return f8c(xb.T.reshape(2, 2, P, -1).transpose(2, 0, 1, 3))

    # W^T pair layout [p, g, t, f]: value W[f, e], e = (2g+t)*128 + p
    def dr_w(W):
        return f8c(np.asarray(W, dtype=np.float32).T
                   .reshape(2, 2, P, E).transpose(2, 0, 1, 3))

    # Wo bf16 by head, rows duplicated to 128 partitions and halved
    # (the duplicated-oT output projection computes 2x): [p, h, j] =
    # Wo[j, h*64 + p%64] / 2
    wo_h = (np.asarray(Wo, dtype=np.float32).T.reshape(H, D, E) / 2.0)
    wo_a = np.ascontiguousarray(
        np.stack([wo_h, wo_h], axis=0)        # [2, H, D, E]
        .transpose(0, 2, 1, 3)                # [2, D, H, E]; p = t*64+d
        .reshape(P, H, E).astype(bf))

    # q scale 1/16: the duplicated-dims scores matmul computes 2x(k.q)
    bq_r = f32c(np.asarray(bq).reshape(FC, P).T / 16.0)
    bk_r = f32c(np.asarray(bk).reshape(FC, P).T)
    bv_a = f32c(bv)
    bo_a = np.asarray(bo, dtype=np.float32)

    xT = [dr_x(x[b]) for b in range(B)]
    xTs = [np.ascontiguousarray(
        t.reshape(P, 2, 2, 4, 1024).transpose(3, 0, 1, 2, 4)) for t in xT]
    wq_a, wk_a, wv_a = dr_w(Wq), dr_w(Wk), dr_w(Wv)

    in_maps = []
    for c in range(8):
        b, r = c // 4, c % 4
        xres = (x[b, r * QR:(r + 1) * QR] + bo_a).reshape(NQC, P, E)
        in_maps.append({
            "xT": xTs[b],
            "xqT": np.ascontiguousarray(xT[b][:, :, :, r * QR:(r + 1) * QR]),
            "xres": f32c(xres.transpose(1, 0, 2)),
            "wq": wq_a, "wk": wk_a, "wv": wv_a, "wo": wo_a,
            "bq": bq_r, "bk": bk_r, "bv": bv_a,
        })
    return in_maps


def assemble(results):
    out = np.empty((B, S, E), dtype=np.float32)
    for c in range(8):
        b, r = c // 4, c % 4
        out[b, r * QR:(r + 1) * QR] = results[c]["out"]
    return out


def kernel(x, Wq, bq, Wk, bk, Wv, bv, Wo, bo):
    in_maps = make_in_maps(x, Wq, bq, Wk, bk, Wv, bv, Wo, bo)
    res = run_spmd(in_maps)
    return assemble(res.results)
